# revision 5
# baseline (speedup 1.0000x reference)
"""Multi-head attention (B=2, S=2048, D=1024, H=16) on 8 Trainium2 cores.

Sharding: 2 batch groups x 4 head-groups. Core c handles batch b=c//4 and
heads [4g, 4g+4) with g=c%4. Each core:
  - transposes its batch's x into xT (din on partitions) via PE transposes,
  - projects qT/kT (head-dims on partitions) and v (natural, 65-stride
    layout with a ones column per head for softmax row sums),
  - computes scores^T = k q^T per head with exp on ACT, attn@v with the
    ones-augmented v so softmax denominators fall out of the same matmul,
  - normalizes via reciprocal + PE rank-1 broadcast,
  - computes its partial out^T = Wo[:, slice] @ attnT_slice,
  - ReduceScatters partials within its batch group (4 cores) so rank g ends
    up with dout rows [256g, 256g+256) of the summed out^T.
Host assembles the 8 [256, 2048] slices into [2, 2048, 1024].

All matmuls run in float32r (TF32-like fast path, 1 cycle/row).
"""

import sys

sys.path.insert(0, "/opt/trn_rl_repo")

import numpy as np

import concourse.bass as bass
import concourse.mybir as mybir
import concourse.tile as tile
from concourse import bacc
from concourse.bass_utils import run_bass_kernel_spmd
from concourse.masks import make_identity

F32 = mybir.dt.float32
F32R = mybir.dt.float32r
AF = mybir.ActivationFunctionType
ALU = mybir.AluOpType

S = 2048          # sequence length per batch
D = 1024          # embed dim
DC = 8            # din chunks of 128
HPC = 4           # heads per core
HD = 64           # head dim
HSL = HPC * HD    # 256: head-dim slice per core
NST = S // 128    # 16 seq tiles
VW = HD + 1       # 65: v block width per head (with ones column)

_NC_CACHE = None


def _alt_copy(nc, i, out, in_):
    """Alternate PSUM->SBUF copies between DVE and ACT to balance load."""
    if i % 2 == 0:
        nc.vector.tensor_copy(out, in_)
    else:
        nc.scalar.copy(out, in_)


def build():
    nc = bacc.Bacc(None, target_bir_lowering=False)

    x = nc.declare_dram_parameter("x", [S, D], F32R, isOutput=False)
    wq = nc.declare_dram_parameter("wq", [HSL, D], F32R, isOutput=False)
    wk = nc.declare_dram_parameter("wk", [HSL, D], F32R, isOutput=False)
    wv = nc.declare_dram_parameter("wv", [HSL, D], F32R, isOutput=False)
    wo = nc.declare_dram_parameter("wo", [D, HSL], F32R, isOutput=False)
    bq = nc.declare_dram_parameter("bq", [HSL], F32, isOutput=False)
    bk = nc.declare_dram_parameter("bk", [HSL], F32, isOutput=False)
    bv = nc.declare_dram_parameter("bv", [HSL], F32, isOutput=False)
    bo = nc.declare_dram_parameter("bo", [HSL], F32, isOutput=False)
    out_slice = nc.declare_dram_parameter("out_slice", [HSL, S], F32, isOutput=True)

    with tile.TileContext(nc) as tc:
        with tc.tile_pool(name="res", bufs=1) as res, \
             tc.tile_pool(name="ld", bufs=2) as ld, \
             tc.tile_pool(name="ptp", bufs=2) as ptp, \
             tc.tile_pool(name="rop", bufs=2) as rop, \
             tc.tile_pool(name="recp", bufs=1) as recp, \
             tc.tile_pool(name="ps", bufs=1, space="PSUM") as ps, \
             tc.tile_pool(name="dram", bufs=1, space="DRAM") as dram:

            # ---- constants / biases ----
            identf = res.tile([128, 128], F32)
            make_identity(nc, identf[:])
            ident = res.tile([128, 128], F32R)
            nc.vector.tensor_copy(ident[:], identf[:])
            ones1 = res.tile([1, 128], F32)
            nc.vector.memset(ones1[:], 1.0)
            onescol = res.tile([128, 1], F32)
            nc.vector.memset(onescol[:], 1.0)
            bq_t = res.tile([128, 2], F32)
            nc.sync.dma_start(out=bq_t[:], in_=bq.rearrange("(i p) -> p i", p=128))
            bk_t = res.tile([128, 2], F32)
            nc.sync.dma_start(out=bk_t[:], in_=bk.rearrange("(i p) -> p i", p=128))
            bo_t = res.tile([128, 2], F32)
            nc.sync.dma_start(out=bo_t[:], in_=bo.rearrange("(i p) -> p i", p=128))
            bva = res.tile([1, HSL], F32)
            nc.sync.dma_start(out=bva[:], in_=bv[None, :])

            # ---- persistent SBUF tensors ----
            xt = res.tile([128, DC * S], F32R)        # x^T, chunk dc at cols dc*S
            wqt = res.tile([128, DC * HSL], F32R)     # Wq^T chunks
            wkt = res.tile([128, DC * HSL], F32R)
            wvt = res.tile([128, DC * HSL], F32R)
            wot = res.tile([128, 2 * D], F32R)        # Wo^T chunks (2 din chunks)
            qt = res.tile([128, 2 * S], F32R)         # q^T (scaled), block h2 at h2*S
            ktt = res.tile([128, 2 * S], F32R)        # k^T
            vt = res.tile([128, NST * HPC * VW], F32R)  # v, 65-stride + ones cols
            at = res.tile([128, 2 * S], F32R)         # normalized attn^T

            rs_in = dram.tile([D, S], F32)
            rs_out = dram.tile([HSL, S], F32)

            cp = 0  # copy-engine alternator

            # ---- phase A: x load + transpose ----
            for st in range(NST):
                xl = ld.tile([128, D], F32R, tag="ld", name=f"xl{st}")
                nc.sync.dma_start(out=xl[:], in_=x[st * 128:(st + 1) * 128, :])
                for dc in range(DC):
                    tp = ps.tile([128, 128], F32R, tag="mm", name=f"tpx{st}_{dc}")
                    nc.tensor.transpose(tp[:], xl[:, dc * 128:(dc + 1) * 128], ident[:])
                    _alt_copy(nc, cp, xt[:, dc * S + st * 128: dc * S + (st + 1) * 128], tp[:]); cp += 1

            # ---- phase B: weight transposes ----
            for wsrc, wdst, nm in ((wq, wqt, "q"), (wk, wkt, "k"), (wv, wvt, "v")):
                for r2 in range(2):
                    wl = ld.tile([128, D], F32R, tag="ld", name=f"wl{nm}{r2}")
                    nc.sync.dma_start(out=wl[:], in_=wsrc[r2 * 128:(r2 + 1) * 128, :])
                    for dc in range(DC):
                        tp = ps.tile([128, 128], F32R, tag="mm", name=f"tpw{nm}{r2}_{dc}")
                        nc.tensor.transpose(tp[:], wl[:, dc * 128:(dc + 1) * 128], ident[:])
                        _alt_copy(nc, cp, wdst[:, dc * HSL + r2 * 128: dc * HSL + r2 * 128 + 128], tp[:]); cp += 1
            for p8 in range(DC):
                wl = ld.tile([128, D], F32R, tag="ld", name=f"wlo{p8}")
                nc.sync.dma_start(out=wl[:, 0:HSL], in_=wo[p8 * 128:(p8 + 1) * 128, :])
                for dc2 in range(2):
                    tp = ps.tile([128, 128], F32R, tag="mm", name=f"tpo{p8}_{dc2}")
                    nc.tensor.transpose(tp[:], wl[:, dc2 * 128: dc2 * 128 + 128], ident[:])
                    _alt_copy(nc, cp, wot[:, dc2 * D + p8 * 128: dc2 * D + (p8 + 1) * 128], tp[:]); cp += 1

            # ---- vt ones columns ----
            vt5 = vt.rearrange("p (s h c) -> p s h c", s=NST, h=HPC)
            nc.vector.tensor_copy(
                vt5[:, :, :, HD:VW], onescol[:].broadcast_to([128, NST, HPC, 1]))

            # ---- phase C: projections ----
            for h2 in range(2):
                for sb4 in range(4):
                    pq = ps.tile([128, 512], F32, tag="mm", name=f"pq{h2}_{sb4}")
                    for dc in range(DC):
                        nc.tensor.matmul(
                            pq[:],
                            wqt[:, dc * HSL + h2 * 128: dc * HSL + h2 * 128 + 128],
                            xt[:, dc * S + sb4 * 512: dc * S + (sb4 + 1) * 512],
                            start=(dc == 0), stop=(dc == DC - 1))
                    nc.vector.tensor_scalar(
                        out=qt[:, h2 * S + sb4 * 512: h2 * S + (sb4 + 1) * 512],
                        in0=pq[:], scalar1=bq_t[:, h2:h2 + 1], scalar2=float(HD) ** -0.5,
                        op0=ALU.add, op1=ALU.mult)
                    pk = ps.tile([128, 512], F32, tag="mm", name=f"pk{h2}_{sb4}")
                    for dc in range(DC):
                        nc.tensor.matmul(
                            pk[:],
                            wkt[:, dc * HSL + h2 * 128: dc * HSL + h2 * 128 + 128],
                            xt[:, dc * S + sb4 * 512: dc * S + (sb4 + 1) * 512],
                            start=(dc == 0), stop=(dc == DC - 1))
                    nc.vector.tensor_scalar(
                        out=ktt[:, h2 * S + sb4 * 512: h2 * S + (sb4 + 1) * 512],
                        in0=pk[:], scalar1=bk_t[:, h2:h2 + 1], scalar2=None, op0=ALU.add)

            for st in range(NST):
                pv = ps.tile([128, HSL], F32, tag="mm", name=f"pv{st}")
                nc.tensor.matmul(pv[:], ones1[:], bva[:], start=True, stop=False)
                for dc in range(DC):
                    nc.tensor.matmul(
                        pv[:],
                        xt[:, dc * S + st * 128: dc * S + (st + 1) * 128],
                        wvt[:, dc * HSL:(dc + 1) * HSL],
                        start=False, stop=(dc == DC - 1))
                nc.vector.tensor_copy(
                    vt5[:, st, :, 0:HD], pv.rearrange("p (h c) -> p h c", h=HPC))

            # ---- phase D: attention ----
            for h in range(HPC):
                h2, r0 = h // 2, (h % 2) * 64
                for qb in range(4):
                    oa = ps.tile([65, 512], F32, tag="oa", name=f"oa{h}_{qb}")
                    for quad in range(4):
                        sc = ps.tile([128, 2048], F32, tag="sc", name=f"sc{h}_{qb}_{quad}")
                        pt_t = ptp.tile([128, 2048], F32R, tag="pt", name=f"pt{h}_{qb}_{quad}")
                        for j in range(4):
                            kt_i = quad * 4 + j
                            nc.tensor.matmul(
                                sc[:, j * 512:(j + 1) * 512],
                                ktt[r0:r0 + 64, h2 * S + kt_i * 128: h2 * S + (kt_i + 1) * 128],
                                qt[r0:r0 + 64, h2 * S + qb * 512: h2 * S + (qb + 1) * 512],
                                start=True, stop=True)
                        nc.scalar.activation(pt_t[:], sc[:], AF.Exp)
                        for j in range(4):
                            kt_i = quad * 4 + j
                            nc.tensor.matmul(
                                oa[:],
                                vt[:, kt_i * HPC * VW + h * VW: kt_i * HPC * VW + (h + 1) * VW],
                                pt_t[:, j * 512:(j + 1) * 512],
                                start=(kt_i == 0), stop=(kt_i == NST - 1))
                    rec_t = recp.tile([1, 512], F32, tag="rec", name=f"rec{h}_{qb}")
                    nc.vector.reciprocal(rec_t[:], oa[64:65, :])
                    pb = ps.tile([64, 512], F32, tag="mm", name=f"pb{h}_{qb}")
                    nc.tensor.matmul(pb[:], ones1[:, 0:64], rec_t[:], start=True, stop=True)
                    rb = recp.tile([64, 512], F32, tag="rb", name=f"rb{h}_{qb}")
                    nc.vector.tensor_copy(rb[:], pb[:])
                    nc.vector.tensor_tensor(
                        out=at[r0:r0 + 64, h2 * S + qb * 512: h2 * S + (qb + 1) * 512],
                        in0=oa[0:64, :], in1=rb[:], op=ALU.mult)

            # ---- phase E: output projection partials + ReduceScatter ----
            for dot in range(DC):
                ro_t = rop.tile([128, S], F32, tag="ro", name=f"ro{dot}")
                for qb in range(4):
                    po = ps.tile([128, 512], F32, tag="mm", name=f"po{dot}_{qb}")
                    for dc2 in range(2):
                        nc.tensor.matmul(
                            po[:],
                            wot[:, dc2 * D + dot * 128: dc2 * D + (dot + 1) * 128],
                            at[:, dc2 * S + qb * 512: dc2 * S + (qb + 1) * 512],
                            start=(dc2 == 0), stop=(dc2 == 1))
                    _alt_copy(nc, cp, ro_t[:, qb * 512:(qb + 1) * 512], po[:]); cp += 1
                nc.sync.dma_start(out=rs_in[dot * 128:(dot + 1) * 128, :], in_=ro_t[:])

            nc.gpsimd.collective_compute(
                "ReduceScatter", ALU.add,
                replica_groups=[[0, 1, 2, 3], [4, 5, 6, 7]],
                ins=[rs_in.opt()], outs=[rs_out.opt()])

            for p2 in range(2):
                rr = rop.tile([128, S], F32, tag="ro", name=f"rr{p2}")
                nc.sync.dma_start(out=rr[:], in_=rs_out[p2 * 128:(p2 + 1) * 128, :])
                nc.vector.tensor_scalar(
                    out=rr[:], in0=rr[:], scalar1=bo_t[:, p2:p2 + 1], scalar2=None,
                    op0=ALU.add)
                nc.sync.dma_start(out=out_slice[p2 * 128:(p2 + 1) * 128, :], in_=rr[:])

    nc.finalize()
    return nc


def _get_nc():
    global _NC_CACHE
    if _NC_CACHE is None:
        _NC_CACHE = build()
    return _NC_CACHE


def make_in_maps(x, Wq, bq, Wk, bk, Wv, bv, Wo, bo):
    """Shard full inputs into 8 per-core input maps."""
    x = np.asarray(x, dtype=np.float32)
    in_maps = []
    for c in range(8):
        b, g = c // 4, c % 4
        sl = slice(g * HSL, (g + 1) * HSL)
        in_maps.append({
            "x": np.ascontiguousarray(x[b]),
            "wq": np.ascontiguousarray(np.asarray(Wq, np.float32)[sl]),
            "wk": np.ascontiguousarray(np.asarray(Wk, np.float32)[sl]),
            "wv": np.ascontiguousarray(np.asarray(Wv, np.float32)[sl]),
            "wo": np.ascontiguousarray(np.asarray(Wo, np.float32)[:, sl]),
            "bq": np.ascontiguousarray(np.asarray(bq, np.float32)[sl]),
            "bk": np.ascontiguousarray(np.asarray(bk, np.float32)[sl]),
            "bv": np.ascontiguousarray(np.asarray(bv, np.float32)[sl]),
            "bo": np.ascontiguousarray(np.asarray(bo, np.float32)[sl]),
        })
    return in_maps


def assemble(results):
    """Gather 8 per-core [256, 2048] out^T slices into [2, 2048, 1024]."""
    out = np.empty((2, S, D), dtype=np.float32)
    for b in range(2):
        out_t = np.concatenate(
            [np.asarray(results[4 * b + g]["out_slice"]) for g in range(4)], axis=0)
        out[b] = out_t.T
    return out


def kernel(x, Wq, bq, Wk, bk, Wv, bv, Wo, bo):
    nc = _get_nc()
    in_maps = make_in_maps(x, Wq, bq, Wk, bk, Wv, bv, Wo, bo)
    res = run_bass_kernel_spmd(nc, in_maps, list(range(8)))
    return assemble(res.results)
